# revision 1
# baseline (speedup 1.0000x reference)
"""Trainium2 Bass kernel for nn_MemoryEfficientAttention (full MHA).

Reference computation (fp32):
    q = split_heads(x @ Wq.T + bq); k, v likewise
    attn = softmax(q @ k.T / sqrt(64))
    out = merge_heads(attn @ v) @ Wo.T + bo

Shapes: B=2, S=4096, D=1024, H=16, head_dim=64.

Sharding across 8 NeuronCores (Megatron attention sharding):
  - 2 heads per core (= 128 of the 1024 projection dims, contiguous slice).
  - Q/K/V projections column-parallel, output projection row-parallel;
    the 8 per-core partial outputs are summed on the host (+ bo).
  - bv never enters the device: softmax rows sum to 1, so its entire effect
    on the output is the constant vector Wo @ bv, added on the host.

Per-core kernel (flash-attention style, nothing S^2-sized touches HBM):
  Phase 1: qT/kT = W_c @ x.T + b_c as fp16 matmuls ([128, S] transposed
           layouts); V projected directly in natural [S, 128] layout
           (x-tile stationary), stored with an interleaved ones-column per
           head (v_aug) that makes the PV matmul accumulate the softmax
           denominator in row 64 for free.
  Phase 2: per (batch, q-chunk of 512): loop over 32 key tiles:
           scoresT[kpos, q] for both heads (row-packed in the PE array,
           they run concurrently), exp split between ScalarE (exact, head
           A's columns) and VectorE (Schraudolph int16 bit-trick exp, head
           B's columns) so neither engine gates the loop; scores run
           DEPTH=2 key tiles ahead of the PV matmuls so the exp latency is
           hidden behind the in-order PE's stream. PV runs in fp8e4
           DoubleRow: exp outputs are written as fp8 into pair slabs
           ([128, 2, 1024]) and one matmul per head contracts a key-tile
           PAIR (256 virtual rows, v stored as [128, NKT/2, 2, 160] fp8
           with ones columns for the denominator). Normalization muls are
           deferred into the middle of the NEXT chunk (and the first
           EXP_FULL_J tiles of each chunk run ACT-only) so the DVE FIFO
           never parks on the denominator DMA roundtrip. Out-proj and
           projection matmuls share one double-buffered PSUM pool so tail
           matmuls don't serialize on their PSUM-release copies.
  Phase 3: out[s, :] = attnT_c.T @ WoT_c in fp32r (natural layout, clean
           DMA out).
"""

import sys

if "/opt/trn_rl_repo" not in sys.path:
    sys.path.insert(0, "/opt/trn_rl_repo")

import numpy as np

B = 2
S_FULL = 4096
D = 1024
H = 16
HD = 64
NCORES = 8
DC = 128          # head dims per core (2 heads x 64)
SCALE = 1.0 / 8.0  # 1/sqrt(64)

# --- softmax-exp engine split (columns of each [128, 1024] scores tile) ---
# ACT does exact exp; DVE + GpSimd compute a Schraudolph fp16 exp
# (i16 = floor(1024*(x*SCALE*log2e + 15 + SIG) + 0.5), bit-viewed as f16).
# The flat half-LSB shift cancels in softmax normalization; the residual
# piecewise-linear bias (~3% max) is confined to EXP_DVE+EXP_GP of every
# 1024 query positions and measurably costs ~5e-3 total rel err.
EXP_ACT = 640
EXP_DVE = 1024 - EXP_ACT
EXP_FULL_J = 8    # first key tiles per chunk: ACT does all 1024 cols, giving
                  # DVE room to clear its chunk-boundary burst off the PV path
NORM_J = 3        # key-tile index at which the previous chunk's deferred
                  # normalization muls are emitted on DVE (inside the
                  # ACT-full window, where DVE is otherwise idle)
SCH_SIG = -0.044
SCH_C1 = float(1024.0 * np.log2(np.e) * SCALE)        # mult constant
SCH_C2 = float(1024.0 * (15.0 + SCH_SIG) + 0.5)       # add constant (+0.5: trunc->round)

# fp8e4m3 variants (PV runs in fp8 DoubleRow: 2 key tiles per matmul).
# TRN fp8e4: bias 7, bit pattern = 8*exp + mant; inf at 0x78=120. Our
# i8 range is [10, 103] so no inf/nan patterns are reachable.
SCH8_SIG = -0.046
SCH8_C1 = float(8.0 * np.log2(np.e) * SCALE)
SCH8_C2 = float(8.0 * (7.0 + SCH8_SIG) + 0.5)
DBG = False


def build_kernel(S=S_FULL):
    """Build the per-core Bass program. Returns the compiled Bacc object."""
    import concourse.bacc as bacc
    import concourse.tile as tile
    from concourse import mybir

    f32 = mybir.dt.float32
    f32r = mybir.dt.float32r
    f16 = mybir.dt.float16
    AF = mybir.ActivationFunctionType

    KT = D // 128       # k-tiles over the projection contraction dim
    SQ = 512            # q-chunk size
    NQC = S // SQ       # q chunks per batch
    NKT = S // 128      # key tiles per batch
    NM = S // 512       # x chunks for projections

    nc = bacc.Bacc("TRN2", target_bir_lowering=False, debug=False,
                   num_devices=NCORES)

    i16 = mybir.dt.int16
    u8 = mybir.dt.uint8
    f8 = mybir.dt.float8e4
    NPAIR = None  # set below once NKT known

    xT = nc.dram_tensor("xT", [B, D, S], f16, kind="ExternalInput").ap()
    wqT = nc.dram_tensor("wqT", [D, DC], f16, kind="ExternalInput").ap()
    wkT = nc.dram_tensor("wkT", [D, DC], f16, kind="ExternalInput").ap()
    wvT = nc.dram_tensor("wvT", [D, DC], f16, kind="ExternalInput").ap()
    woT = nc.dram_tensor("woT", [DC, D], f16, kind="ExternalInput").ap()
    bq = nc.dram_tensor("bq", [DC], f32, kind="ExternalInput").ap()
    bk = nc.dram_tensor("bk", [DC], f32, kind="ExternalInput").ap()
    ones = nc.dram_tensor("ones", [128, 32], f16, kind="ExternalInput").ap()
    part = nc.dram_tensor("part", [B, S, D], f16, kind="ExternalOutput").ap()
    if DBG:
        dbg_v = nc.dram_tensor("dbg_v", [128, S // 256, 2, 160], f8,
                               kind="ExternalOutput").ap()
        dbg_e = nc.dram_tensor("dbg_e", [128, 2, 1024], f8,
                               kind="ExternalOutput").ap()
        dbg_a = nc.dram_tensor("dbg_a", [2, 128, 512], f32,
                               kind="ExternalOutput").ap()

    with tile.TileContext(nc) as tc:
        with (
            tc.tile_pool(name="consts", bufs=1) as consts,
            tc.tile_pool(name="xt", bufs=4) as xt_pool,
            tc.tile_pool(name="qkv", bufs=2) as qkv_pool,
            tc.tile_pool(name="exp", bufs=4) as exp_pool,
            tc.tile_pool(name="att", bufs=2) as att_pool,
            tc.tile_pool(name="small", bufs=3) as small_pool,
            tc.tile_pool(name="outs", bufs=6) as out_pool,
            tc.tile_pool(name="bounce", bufs=2, space="DRAM") as dram_pool,
            tc.tile_pool(name="ps_mix", bufs=2, space="PSUM") as ps_mix,
            tc.tile_pool(name="ps_scores", bufs=2, space="PSUM") as ps_scores,
            tc.tile_pool(name="ps_acc", bufs=2, space="PSUM") as ps_acc,
        ):
            # ---- constants ----
            wq_sb = consts.tile([128, KT, DC], f16)
            wk_sb = consts.tile([128, KT, DC], f16)
            wv_sb = consts.tile([128, KT, DC], f16)
            wo_sb = consts.tile([128, D], f16)
            bq_sb = consts.tile([128, 1], f32)
            bk_sb = consts.tile([128, 1], f32)

            # tiny dummy exp so the ACT table set loads during startup DMAs
            warm = consts.tile([128, 1], f32)
            nc.vector.memset(warm[:], 0.0)
            nc.scalar.activation(warm[:], warm[:], AF.Exp, scale=1.0)

            for w_sb, w_dram in ((wq_sb, wqT), (wk_sb, wkT), (wv_sb, wvT)):
                nc.gpsimd.dma_start(
                    out=w_sb[:],
                    in_=w_dram.rearrange("(kt p) m -> p kt m", p=128),
                )
            nc.gpsimd.dma_start(out=wo_sb[:], in_=woT)
            for b_sb, b_dram in ((bq_sb, bq), (bk_sb, bk)):
                nc.gpsimd.dma_start(out=b_sb[:], in_=b_dram.rearrange("(p o) -> p o", o=1))

            state = [None, None]  # per-batch dict of tiles

            def alloc_batch(b):
                qT_sb = qkv_pool.tile([128, S], f16, tag="qT", name=f"qT_{b}")
                kT_sb = qkv_pool.tile([128, S], f16, tag="kT", name=f"kT_{b}")
                # v in fp8e4 DoubleRow pair layout: per key-tile PAIR jp,
                # slab c = key tile 2jp+c: [vA(64) | 1 | pad | vB(64) | 1]
                v_sb = qkv_pool.tile([128, NKT // 2, 2, 160], f8, tag="v",
                                     name=f"v_{b}")
                if DBG:
                    nc.vector.memset(v_sb[:], 0.0)
                nc.vector.memset(v_sb[:, :, :, 64:65], 1.0)
                nc.vector.memset(v_sb[:, :, :, 144:145], 1.0)
                attT_sb = att_pool.tile([128, S], f16, tag="attT",
                                        name=f"attT_{b}")
                den_dram = dram_pool.tile([2, S], f16, tag="den",
                                          name=f"den_{b}")
                state[b] = dict(qT=qT_sb, kT=kT_sb, v=v_sb, attT=attT_sb,
                                den=den_dram)

            def emit_proj_chunk(b, m):
                """Projections for x columns [m*512, (m+1)*512) of batch b."""
                st = state[b]
                xt = xt_pool.tile([128, KT, 512], f16, tag="xt",
                                  name=f"xt_{b}_{m}")
                xsrc = xT[b][:, m * 512:(m + 1) * 512].rearrange(
                    "(kt p) s -> p kt s", p=128)
                half = KT // 2
                nc.sync.dma_start(out=xt[:, 0:half, :], in_=xsrc[:, 0:half, :])
                nc.gpsimd.dma_start(out=xt[:, half:, :], in_=xsrc[:, half:, :])
                for w_sb, b_sb, dst in (
                    (wk_sb, bk_sb, st["kT"]),
                    (wq_sb, bq_sb, st["qT"]),
                ):
                    ps = ps_mix.tile([128, 512], f32, tag="mix",
                                      name=f"ps_{b}_{m}")
                    for j in range(KT):
                        nc.tensor.matmul(
                            ps[:],
                            lhsT=w_sb[:, j, :],
                            rhs=xt[:, j, :],
                            start=(j == 0),
                            stop=(j == KT - 1),
                        )
                    nc.vector.tensor_scalar_add(
                        dst[:, m * 512:(m + 1) * 512], ps[:], b_sb[:],
                    )
                # V in natural layout: x-tile stationary, Wv moving.
                for t in range(4):
                    psv = ps_mix.tile([128, 512], f32, tag="mix",
                                       name=f"psv_{b}_{m}_{t}")
                    for j in range(KT):
                        nc.tensor.matmul(
                            psv[:, 0:DC],
                            lhsT=xt[:, j, t * 128:(t + 1) * 128],
                            rhs=wv_sb[:, j, :],
                            start=(j == 0),
                            stop=(j == KT - 1),
                        )
                    kt_idx = m * 4 + t
                    jp, cc = kt_idx // 2, kt_idx % 2
                    with nc.allow_low_precision(reason="fp8 PV operand"):
                        nc.vector.tensor_copy(
                            state[b]["v"][:, jp, cc, 0:64], psv[:, 0:64])
                        nc.vector.tensor_copy(
                            state[b]["v"][:, jp, cc, 80:144], psv[:, 64:128])

            def emit_attn(b, qc, fillers=None, drain_hook=None):
                """Attention for one q-chunk of 512 rows.

                Software-pipelined: scores for key tile j+1 are emitted
                (and thus queued on the in-order PE) BEFORE the PV matmuls
                of tile j, so the PE streams scores while the three exp
                engines (ACT/DVE/GpSimd, split by column range) catch up.
                """
                st = state[b]
                qT_sb, kT_sb, v_sb = st["qT"], st["kT"], st["v"]
                attT_sb, den_dram = st["attT"], st["den"]
                q0, q1 = qc * SQ, (qc + 1) * SQ
                acc_a = ps_acc.tile([128, SQ], f32, tag="acc",
                                    name=f"acca_{b}_{qc}")
                acc_b = ps_acc.tile([128, SQ], f32, tag="acc",
                                    name=f"accb_{b}_{qc}")
                accs = [acc_a, acc_b]

                def emit_scores(j, ex8):
                    """Scores pair for key tile j; exp written into slab
                    j%2 of the fp8 pair tile ex8."""
                    k0, k1 = j * 128, (j + 1) * 128
                    pss = ps_scores.tile([128, 2 * SQ], f32, tag="scores",
                                         name=f"pss_{b}_{qc}_{j}")
                    for hh in range(2):
                        nc.tensor.matmul(
                            pss[:, hh * SQ:(hh + 1) * SQ],
                            lhsT=kT_sb[hh * 64:(hh + 1) * 64, k0:k1],
                            rhs=qT_sb[hh * 64:(hh + 1) * 64, q0:q1],
                            start=True, stop=True,
                        )
                    sl = j % 2
                    with nc.allow_low_precision(reason="fp8 softmax weights"):
                        if j < full_j:
                            nc.scalar.activation(ex8[:, sl, :], pss[:],
                                                 AF.Exp, scale=SCALE)
                        else:
                            c0 = EXP_ACT
                            nc.scalar.activation(ex8[:, sl, 0:c0],
                                                 pss[:, 0:c0],
                                                 AF.Exp, scale=SCALE)
                            nc.vector.tensor_scalar(
                                ex8[:, sl, c0:].bitcast(u8), pss[:, c0:],
                                SCH8_C1, SCH8_C2,
                                op0=mybir.AluOpType.mult,
                                op1=mybir.AluOpType.add)

                def emit_pv(jp, ex8):
                    """fp8 DoubleRow PV: one matmul per head contracts the
                    key-tile PAIR (2jp, 2jp+1) = 256 virtual rows."""
                    for hh in range(2):
                        base = hh * 80
                        nc.tensor.matmul(
                            accs[hh][0:65, :],
                            lhsT=v_sb[:, jp, :, base:base + 65],
                            rhs=ex8[:, :, hh * SQ:(hh + 1) * SQ],
                            start=(jp == 0), stop=(jp == NKT // 2 - 1),
                            perf_mode=mybir.MatmulPerfMode.DoubleRow,
                        )

                DEPTH_PAIRS = 2  # PV pairs lag the scores by 2 pairs (4 tiles)
                pend_pv = []
                ex_cur = None
                norm_j = min(NORM_J, NKT - 1)
                # filler work (b1 proj chunks) is emitted INSIDE the
                # ACT-full window so its DVE ops land where DVE has no
                # exp work; such chunks get a longer ACT-full prefix.
                full_j = EXP_FULL_J if not fillers else EXP_FULL_J + 4
                for j in range(NKT):
                    if j == norm_j:
                        flush_norms()
                    if fillers and j in (1, 3):
                        fillers.pop(0)()
                    if j == 5 and drain_hook is not None:
                        drain_hook()
                    if j % 2 == 0:
                        ex_cur = exp_pool.tile([128, 2, 2 * SQ], f8,
                                               tag="exp",
                                               name=f"ex_{b}_{qc}_{j // 2}")
                    emit_scores(j, ex_cur)
                    if DBG and b == 0 and qc == 0 and j == 1:
                        nc.sync.dma_start(out=dbg_e[:], in_=ex_cur[:])
                    if j % 2 == 1:
                        pend_pv.append((j // 2, ex_cur))
                        if len(pend_pv) > DEPTH_PAIRS:
                            emit_pv(*pend_pv.pop(0))
                for args in pend_pv:
                    emit_pv(*args)
                if DBG and b == 0 and qc == 0:
                    nc.sync.dma_start(out=dbg_v[:], in_=v_sb[:])
                    for hh in range(2):
                        a_sb = out_pool.tile([128, SQ], f32, tag="dbga",
                                             name=f"dbga_{hh}")
                        nc.vector.memset(a_sb[:], 0.0)
                        nc.vector.tensor_copy(a_sb[0:65, :], accs[hh][0:65, :])
                        nc.sync.dma_start(out=dbg_a[hh], in_=a_sb[:])

                # fast PSUM release: copy raw output + reciprocal denominator
                for hh in range(2):
                    nc.vector.tensor_copy(
                        attT_sb[hh * 64:(hh + 1) * 64, q0:q1],
                        accs[hh][0:64, :],
                    )
                    dn = small_pool.tile([1, SQ], f32, tag="dn",
                                         name=f"dn_{b}_{qc}_{hh}")
                    nc.vector.tensor_copy(dn[:], accs[hh][64:65, :])
                    rcp = small_pool.tile([1, SQ], f32, tag="rcp",
                                          name=f"rcp_{b}_{qc}_{hh}")
                    nc.vector.reciprocal_approx_fast(rcp[:], dn[:])
                    rcp16 = small_pool.tile([1, SQ], f16, tag="rcp16",
                                            name=f"rcp16_{b}_{qc}_{hh}")
                    with nc.allow_low_precision(reason="fp16 softmax recip"):
                        nc.vector.tensor_copy(rcp16[:], rcp[:])
                    nc.sync.dma_start(out=den_dram[hh, q0:q1], in_=rcp16[:])

                # broadcast the reciprocal + normalize; resolves during the
                # next chunk's attention
                for hh in range(2):
                    bc = small_pool.tile([128, SQ], f16, tag="bcast",
                                         name=f"bc_{b}_{qc}_{hh}")
                    bch = bc[hh * 64:(hh + 1) * 64, :]
                    rd = den_dram[hh, q0:q1]
                    bcast_src = rd.__class__(
                        tensor=rd.tensor, offset=rd.offset,
                        ap=[[0, 64]] + list(rd.ap),
                    )
                    nc.sync.dma_start(out=bch, in_=bcast_src)

                    def mul_closure(hh=hh, bch=bch, attT_sb=attT_sb,
                                    q0=q0, q1=q1):
                        nc.vector.tensor_mul(
                            attT_sb[hh * 64:(hh + 1) * 64, q0:q1],
                            attT_sb[hh * 64:(hh + 1) * 64, q0:q1],
                            bch,
                        )
                    pending_norm.append(mul_closure)

            def emit_tail(b, qc, use_act=False, copies_on_act=True):
                """Output projection for a q-chunk, emitted one chunk late
                so the normalization chain has already resolved and the
                in-order PE never stalls on it."""
                st = state[b]
                attT_sb = st["attT"]
                q0, q1 = qc * SQ, (qc + 1) * SQ
                # output projection for this q-chunk's 4 row-tiles
                for sti in range(SQ // 128):
                    s0 = q0 + sti * 128
                    s1 = s0 + 128
                    for oc in range(D // 512):
                        pso = ps_mix.tile([128, 512], f32, tag="mix",
                                          name=f"pso_{b}_{qc}_{sti}_{oc}")
                        nc.tensor.matmul(
                            pso[:],
                            lhsT=attT_sb[:, s0:s1],
                            rhs=wo_sb[:, oc * 512:(oc + 1) * 512],
                            start=True, stop=True,
                        )
                        ob = out_pool.tile([128, 512], f16, tag="ob",
                                           name=f"ob_{b}_{qc}_{sti}_{oc}")
                        with nc.allow_low_precision(reason="f16 partial out"):
                            if use_act or (oc == 1 and copies_on_act):
                                nc.scalar.copy(ob[:], pso[:])
                            else:
                                nc.vector.tensor_copy(ob[:], pso[:])
                        nc.sync.dma_start(
                            out=part[b, s0:s1, oc * 512:(oc + 1) * 512],
                            in_=ob[:],
                        )

            # ---- emission schedule: batch 1's projections are interleaved
            # into batch 0's late attention so ScalarE never drains ----
            TAIL_DELAY = 1
            pending = []
            pending_norm = []

            def flush_norms():
                while pending_norm:
                    pending_norm.pop(0)()

            def drain_tails(limit, use_act=False, copies_on_act=True):
                if limit == 0:
                    flush_norms()
                while len(pending) > limit:
                    emit_tail(*pending.pop(0), use_act=use_act,
                              copies_on_act=copies_on_act)

            alloc_batch(0)
            for m in range(NM):
                emit_proj_chunk(0, m)
            half = NQC // 2
            done_m = 0
            for qc in range(NQC):
                fill = []
                if NQC >= 4 and qc >= half - 1 and done_m < NM:
                    if state[1] is None:
                        alloc_batch(1)
                    for _ in range(2):
                        if done_m < NM:
                            fill.append(
                                lambda m=done_m: emit_proj_chunk(1, m))
                            done_m += 1
                emit_attn(0, qc, fillers=fill,
                          drain_hook=lambda: drain_tails(0))
                pending.append((0, qc))
            if state[1] is None:
                alloc_batch(1)
            while done_m < NM:
                emit_proj_chunk(1, done_m)
                done_m += 1
            for qc in range(NQC):
                emit_attn(1, qc, drain_hook=lambda: drain_tails(0))
                pending.append((1, qc))
            drain_tails(0, use_act=True)

    nc.compile()
    return nc


def shard_inputs(x, Wq, bq, Wk, bk, Wv, bv, Wo, bo, S=S_FULL):
    """Host-side sharding: returns list of 8 per-core input dicts."""
    x = np.asarray(x, dtype=np.float32)
    xT = np.ascontiguousarray(x.transpose(0, 2, 1)).astype(np.float16)  # [B, D, S]
    in_maps = []
    for c in range(NCORES):
        sl = slice(c * DC, (c + 1) * DC)
        in_maps.append({
            "xT": xT,
            "wqT": np.ascontiguousarray(np.asarray(Wq)[sl, :].T).astype(np.float16),
            "wkT": np.ascontiguousarray(np.asarray(Wk)[sl, :].T).astype(np.float16),
            "wvT": np.ascontiguousarray(np.asarray(Wv)[sl, :].T).astype(np.float16),
            "woT": np.ascontiguousarray(np.asarray(Wo)[:, sl].T).astype(np.float16),
            "bq": np.ascontiguousarray(np.asarray(bq)[sl], dtype=np.float32),
            "bk": np.ascontiguousarray(np.asarray(bk)[sl], dtype=np.float32),
            "ones": np.ones((128, 32), dtype=np.float16),
        })
    return in_maps


_NC_CACHE = {}


def _get_nc(S=S_FULL):
    if S not in _NC_CACHE:
        _NC_CACHE[S] = build_kernel(S)
    return _NC_CACHE[S]


def kernel(x, Wq, bq, Wk, bk, Wv, bv, Wo, bo, _trace=False, _trace_cores=None):
    from concourse import bass_utils

    nc = _get_nc(S_FULL)
    in_maps = shard_inputs(x, Wq, bq, Wk, bk, Wv, bv, Wo, bo)
    kwargs = {}
    if _trace:
        kwargs = dict(trace=True, trace_cores=_trace_cores or [0])
    res = bass_utils.run_bass_kernel_spmd(
        nc, in_maps, core_ids=list(range(NCORES)), **kwargs)
    out = np.zeros((B, S_FULL, D), dtype=np.float32)
    for c in range(NCORES):
        out += res.results[c]["part"].astype(np.float32)
    # bv is folded out of the device kernel: softmax rows sum to one, so its
    # contribution to the output is the constant Wo @ bv. Add it with bo here.
    bias = (np.asarray(Wo, dtype=np.float64) @ np.asarray(bv, dtype=np.float64)
            + np.asarray(bo, dtype=np.float64))
    out += bias.astype(np.float32)[None, None, :]
    if _trace:
        kernel._last_results = res
    return out

